# revision 35
# baseline (speedup 1.0000x reference)
"""Trainium2 Bass kernel for nn_Conv2d_uint8_custom (dynamic uint8 quant + LUT conv).

Semantics (matches reference.py):
  qf = clip(round(x/scale_f) + zero_f, 0, 255)          (per-tensor dynamic quant)
  qw = clip(round(w/scale_w) + zero_w, 0, 255)
  acc[b,o,l] = sum_k lut[qf_patch, qw] = sum_k qf*qw     (lut is an exact product table)
  out = (acc - zero_f * qw_sum[o]) * scale_f * scale_w + bias[o]

Strategy (v5):
  * batch-parallel across 8 cores (2 images per core)
  * ALL quantization on host (exact fp32 replication of the reference);
    device receives pre-quantized bf16 features (ints 0..255, exact)
  * ONE feature plane per image: [flat(64ch) | up1(64ch)] in the padded
    58x58 geometry. The kh0/kh1 taps pair at K=128 for every kw via the
    column offset (0/1/2) alone; the three kh2 taps are K=64 matmuls
    that exist on BOTH halves (flat rows rt+2 / up1 rows rt+1), so two
    of them run CONCURRENTLY in disjoint PE row-groups (32x32 subarray
    tiling), and the third pairs with the adjacent tile's. Net: 4.5
    matmul slots per tile -- same PE time as the 2-plane layout with
    HALF the feature DMA (1.72MB/core) and half the weight bytes
    (one shared [128,5,128] table for both images).
  * weights ride the scalar HW DGE ring head so the first LDWEIGHTS
    gate clears ~2us earlier than the software gpsimd ring
  * PE warmup matmuls sized to bridge preamble -> first-data with no gap
  * epilogue scale+bias into bf16 (vector/scalar alternating; the last
    tile is split across both), stores batched 3+3+1 tiles per image
    with the final small store on the idle sync ring
  * host converts bf16 output back to fp32
"""

import numpy as np
import ml_dtypes
from contextlib import ExitStack

import concourse.bass as bass
import concourse.tile as tile
from concourse import bacc, mybir


def _ensure_axon_ntff_hook():
    """This image's `antenv` lacks `axon_hooks`, which bass_utils imports
    unconditionally when tracing under axon. Provide it (backed by the ctypes
    NTFF hook from trn_agent_boot when available, else None so concourse
    degrades to an untraced run)."""
    import sys, types

    if "antenv.axon_hooks" in sys.modules:
        return
    try:
        import antenv
    except ImportError:
        return
    mod = types.ModuleType("antenv.axon_hooks")
    hook = [None]
    try:
        from trn_agent_boot.trn_boot import _ntff_profile_via_ctypes

        hook[0] = _ntff_profile_via_ctypes("/opt/axon/libaxon_pjrt.so")
    except Exception:
        pass
    mod.get_axon_ntff_profile_hook = lambda: hook[0]
    mod.set_axon_ntff_profile_hook = lambda h: hook.__setitem__(0, h)
    sys.modules["antenv.axon_hooks"] = mod
    antenv.axon_hooks = mod


_ensure_axon_ntff_hook()

N_CORES = 8
B, C, H, W = 16, 64, 56, 56
O = 128
IMG_PER_CORE = B // N_CORES  # 2
L = H * W                    # 3136
HP, WP = H + 2, W + 2        # 58, 58 (zero-padded layout)
LP = HP * WP                 # 3364
TILE_ROWS = 8
NT = H // TILE_ROWS          # 7 output tiles per image
NCOL = TILE_ROWS * W         # 448 columns per tile (one PSUM bank)
N_WARM = 7                   # PE p-state warmup matmuls (big, 256 cols)
N_WARM_SMALL = 26            # trailing fine-grained warmups (64-col bridge)
WARM_COLS = 256

FP32 = mybir.dt.float32
BF16 = mybir.dt.bfloat16

# feature-plane load chunks (padded-row ranges); first small so tile 0's
# data lands as early as possible. Each DMA's completion semaphore is a
# stream of increments serialized per ring (~0.5-1us lag), so chunk
# count per ring stays low
CHUNKS = [(0, 10), (10, 26), (26, 42), (42, 58)]

_NC = None


def _build_nc():
    nc = bacc.Bacc(
        "TRN2",
        debug=False,
        enable_asserts=False,
        num_devices=N_CORES,
        enable_partition_id=False,
    )
    fq_d = nc.dram_tensor("fq", [4, 128, LP], BF16, kind="ExternalInput").ap()
    wq_d = nc.dram_tensor("wq", [128, 5, 128], BF16, kind="ExternalInput").ap()
    qp_d = nc.dram_tensor("qp", [128, 2], FP32, kind="ExternalInput").ap()
    out_d = nc.dram_tensor(
        "out", [IMG_PER_CORE, O, L], BF16, kind="ExternalOutput"
    ).ap()

    with tile.TileContext(nc) as tc:
        with ExitStack() as ctx:
            _body(ctx, tc, fq_d, wq_d, qp_d, out_d)
    nc.compile()
    return nc


def _body(ctx, tc, fq_d, wq_d, qp_d, out_d):
    nc = tc.nc
    A = mybir.AluOpType
    ID = mybir.ActivationFunctionType.Identity
    consts = ctx.enter_context(tc.tile_pool(name="consts", bufs=1))
    fpool = ctx.enter_context(tc.tile_pool(name="feat", bufs=1))
    opool = ctx.enter_context(tc.tile_pool(name="osb", bufs=6))
    ppool = ctx.enter_context(tc.tile_pool(name="acc", bufs=7, space="PSUM"))
    wpool = ctx.enter_context(tc.tile_pool(name="warm", bufs=1, space="PSUM"))

    warm = consts.tile([128, WARM_COLS], BF16)
    wq = consts.tile([128, 5, 128], BF16)
    qp = consts.tile([128, 2], FP32)
    F0 = fpool.tile([128, LP], BF16, name="F0")
    F1 = fpool.tile([128, LP], BF16, name="F1")
    M0 = fpool.tile([128, LP], BF16, name="M0")
    M1 = fpool.tile([128, LP], BF16, name="M1")

    # warm-tile memset on the vector engine (free until epilogues start)
    nc.vector.memset(warm[:], 0.0)

    def ld(eng, T, ci, plane):
        a, b = CHUNKS[ci]
        eng.dma_start(T[:, a * WP : b * WP], fq_d[plane, :, a * WP : b * WP])

    # ring layout (per-ring FIFO == stream priority): the two HW rings
    # carry PURE feature chains (M on sync, F on scalar) so each chain's
    # chunk semaphores pace with its own drain and nothing (wq/qp tiny-
    # DMA sem streams) wedges in front of the early chunks; wq/qp ride
    # the software gpsimd ring (multi-us slack before first use)
    #   sync:   M0 c0..c3, M1 c0..c3, [img0_s2, img1_s2 stores]
    #   scalar: F0 c0..c3, F1 c0..c3, [img1_s1 store]
    #   gpsimd (slow SW ring): wq, qp + bulk stores with slack
    nc.gpsimd.dma_start(wq[:], wq_d[:])
    ld(nc.sync, M0, 0, 2)
    ld(nc.scalar, F0, 0, 0)
    nc.gpsimd.dma_start(qp[:], qp_d[:])
    for ci in range(1, 4):
        ld(nc.sync, M0, ci, 2)
    for ci in range(1, 4):
        ld(nc.scalar, F0, ci, 0)
    for ci in range(4):
        ld(nc.sync, M1, ci, 3)
    for ci in range(4):
        ld(nc.scalar, F1, ci, 1)

    # PE p-state warmup: one PSUM accumulation group of dummy matmuls (no
    # WAW sems) bridges from the preamble to first-data with no idle gap.
    pw = wpool.tile([128, WARM_COLS], FP32, name="pw", tag="pw")
    n_all = N_WARM + N_WARM_SMALL
    for k in range(n_all):
        cols = WARM_COLS if k < N_WARM else 64
        nc.tensor.matmul(
            pw[:, 0:cols], warm[:, 0:128], warm[:, 0:cols],
            start=(k == 0), stop=(k == n_all - 1), skip_group_check=True,
        )

    views = {}

    def plane_view(img):
        if img not in views:
            F = F0 if img == 0 else F1
            M = M0 if img == 0 else M1
            views[img] = (
                F[:].rearrange("p (r c) -> p r c", c=WP),
                M[:].rearrange("p (r c) -> p r c", c=WP),
            )
        return views[img]

    psums = {}

    def gemm_k128(img, t, g):
        """K=128 slot g of tile t. g 0..2: (kh0 via flat + kh1 via up1)
        on the F plane at col offset g. g==3: (kh2,kw0)+(kh2,kw1) on the
        M plane ([flat | left1]) at rows rt+2. g==0 opens the tile's
        psum group. These full-array matmuls also serve as row-group
        BARRIERS around the concurrent K=64 bracket (a full-K matmul
        must wait for all in-flight row-group users, serializing
        same-bank hazards -- unbracketed chains of alternating row-group
        K=64s fault the device)."""
        fv, mv = plane_view(img)
        if g == 0:
            psums[(img, t)] = ppool.tile(
                [128, NCOL], FP32, name=f"ps{img}_{t}", tag="ps"
            )
        ps = psums[(img, t)]
        rt = TILE_ROWS * t
        if g < 3:
            src = fv[:, rt : rt + TILE_ROWS, g : g + W]
        else:
            src = mv[:, rt + 2 : rt + 10, 0:W]
        nc.tensor.matmul(
            ps[:], wq[:, g, :], src,
            start=(g == 0), stop=False, skip_group_check=True,
        )

    def tap9_lo(img, t, stop=True):
        """(kh2,kw2) via the M flat half (rows rt+2, cols+2): row-group 0."""
        _, mv = plane_view(img)
        ps = psums[(img, t)]
        rt = TILE_ROWS * t
        nc.tensor.matmul(
            ps[:], wq[0:64, 4, :], mv[0:64, rt + 2 : rt + 10, 2 : 2 + W],
            start=False, stop=stop, skip_group_check=True,
        )

    def tap9_hi(img, t, stop=True):
        """(kh2,kw2) via the M left1 half (rows rt+2, cols+1): row-group 64."""
        _, mv = plane_view(img)
        ps = psums[(img, t)]
        rt = TILE_ROWS * t
        nc.tensor.matmul(
            ps[:], wq[64:128, 4, :], mv[64:128, rt + 2 : rt + 10, 1 : 1 + W],
            start=False, stop=stop, skip_group_check=True,
        )

    # output staging + stores. per image: batches (t0-2), (t3-5), (t6);
    # the very last store (img1 t6) is small and rides the by-then idle
    # sync ring so its completion semaphore lands asap.
    STORE_ENG = {
        (0, 0): nc.gpsimd, (0, 1): nc.gpsimd, (0, 2): nc.sync,
        (1, 0): nc.gpsimd, (1, 1): nc.scalar, (1, 2): nc.sync,
    }
    obuf = {}

    def epilogue(img, t, ps):
        bi, j = t // 3, t % 3
        nb = 3 if bi < 2 else 1
        if j == 0:
            obuf[img] = opool.tile([128, nb * NCOL], BF16, name=f"o{img}_{bi}")
        dst = obuf[img][:, j * NCOL : (j + 1) * NCOL]
        if (img * NT + t) % 2 == 0:
            nc.scalar.activation(dst, ps[:], ID, bias=qp[:, 0:1], scale=qp[:, 1:2])
        else:
            nc.vector.tensor_scalar(
                dst, ps[:], qp[:, 1:2], qp[:, 0:1], op0=A.mult, op1=A.add
            )
        if j == nb - 1:
            c0 = bi * 3 * NCOL
            STORE_ENG[(img, bi)].dma_start(
                out_d[img, :, c0 : c0 + nb * NCOL], obuf[img][:, 0 : nb * NCOL]
            )

    # Software-pipelined schedule over the 14 tiles (img-major order).
    # Per tile: 4 K=128 slots + half of one K=64 bracket. The two
    # (kh2,kw2) K=64 matmuls of a tile pair run as ONE concurrent
    # cross-bank bracket [rg0 on psX || rg64 on psY], bracketed by
    # full-K=128 matmuls of a LATER tile as row-group barriers.
    TILES = [(0, t) for t in range(NT)] + [(1, t) for t in range(NT)]

    def G(i, g):
        if i < len(TILES):
            gemm_k128(TILES[i][0], TILES[i][1], g)

    G(0, 0); G(0, 1); G(0, 2); G(0, 3)
    G(1, 0)
    for k in range(7):
        X, Y = TILES[2 * k], TILES[2 * k + 1]
        G(2 * k + 1, 1); G(2 * k + 1, 2); G(2 * k + 1, 3)
        tap9_lo(*X)                              # bracket: rg0 on psX
        tap9_hi(*Y)                              #          rg64 on psY
        G(2 * k + 2, 0)                          # barrier + open next tile
        epilogue(*X, psums[X])
        epilogue(*Y, psums[Y])
        G(2 * k + 2, 1); G(2 * k + 2, 2); G(2 * k + 2, 3)
        G(2 * k + 3, 0)


def _prep_host(x, weight, bias):
    """Exact fp32 replication of the reference's quantization arithmetic
    (numpy and jax-on-cpu are both IEEE fp32, round-half-even), then pack
    the padded bf16 feature planes [flat|up1], the shared bf16 weight
    table, and the folded epilogue scale/bias."""
    f = np.float32
    mx, mn = f(x.max()), f(x.min())
    scale_f = f((mx - mn) / f(255.0))
    zero_f = f(-np.round(mn / scale_f))
    qf = np.clip(
        np.round(x.astype(np.float32) / scale_f) + zero_f, 0.0, 255.0
    ).astype(ml_dtypes.bfloat16)  # exact small ints

    mw, nw = f(weight.max()), f(weight.min())
    scale_w = f((mw - nw) / f(255.0))
    zero_w = f(-np.round(nw / scale_w))
    qw = np.clip(
        np.round(weight.astype(np.float32) / scale_w) + zero_w, 0.0, 255.0
    ).astype(np.float32)  # exact small ints

    s_tot = f(scale_f * scale_w)
    qw_sum = qw.reshape(O, -1).sum(axis=1, dtype=np.float64)
    bias_eff = (
        bias.astype(np.float64) - np.float64(zero_f) * qw_sum * np.float64(s_tot)
    ).astype(np.float32)
    qp = np.zeros((128, 2), np.float32)
    qp[:, 0] = bias_eff
    qp[:, 1] = s_tot

    # padded features [B, C, 58*58] + the up1 (flat +58) shifted variant
    pad = np.zeros((B, C, HP, WP), ml_dtypes.bfloat16)
    pad[:, :, 1 : 1 + H, 1 : 1 + W] = qf
    flat = pad.reshape(B, C, LP)
    shU = np.zeros_like(flat)
    shU[:, :, : LP - WP] = flat[:, :, WP:]

    shL = np.zeros_like(flat)
    shL[:, :, : LP - 1] = flat[:, :, 1:]

    # per-core planes [4, 128, LP]: F = [flat|up1] (kh0/kh1 pairs at col
    # offsets 0/1/2), M = [flat|left1] (kh2 pairs at rows rt+2)
    fq_cores = []
    for c in range(N_CORES):
        i0, i1 = 2 * c, 2 * c + 1
        p_f0 = np.concatenate([flat[i0], shU[i0]], axis=0)
        p_f1 = np.concatenate([flat[i1], shU[i1]], axis=0)
        p_m0 = np.concatenate([flat[i0], shL[i0]], axis=0)
        p_m1 = np.concatenate([flat[i1], shL[i1]], axis=0)
        fq_cores.append(
            np.ascontiguousarray(np.stack([p_f0, p_f1, p_m0, p_m1]))
        )

    # shared weight table [128 (K), 5 (slot), 128 (O)]:
    #   s0: (kh0,kw0) lo | (kh1,kw0) hi     s1: (kh0,kw1) | (kh1,kw1)
    #   s2: (kh0,kw2) | (kh1,kw2)           s3: (kh2,kw0) lo | (kh2,kw1) hi
    #   s4: (kh2,kw2) on both halves (tiles use one half each, paired)
    qwT = qw.transpose(2, 3, 1, 0)  # [kh, kw, C, O]
    wqa = np.zeros((128, 5, 128), np.float32)
    for kw in range(3):
        wqa[0:64, kw] = qwT[0, kw]
        wqa[64:128, kw] = qwT[1, kw]
    wqa[0:64, 3] = qwT[2, 0]
    wqa[64:128, 3] = qwT[2, 1]
    wqa[0:64, 4] = qwT[2, 2]
    wqa[64:128, 4] = qwT[2, 2]
    return fq_cores, wqa.astype(ml_dtypes.bfloat16), qp


def build():
    global _NC
    if _NC is None:
        _NC = _build_nc()
    return _NC


LAST_RESULT = None


def kernel(x, weight, bias, lut):
    global LAST_RESULT
    from concourse.bass_utils import run_bass_kernel_spmd

    x = np.asarray(x, dtype=np.float32)
    weight = np.asarray(weight, dtype=np.float32)
    bias = np.asarray(bias, dtype=np.float32)

    fq_cores, wq, qp = _prep_host(x, weight, bias)
    nc = build()
    in_maps = [
        {"fq": fq_cores[c], "wq": wq, "qp": qp} for c in range(N_CORES)
    ]

    res = run_bass_kernel_spmd(nc, in_maps, core_ids=list(range(N_CORES)))
    LAST_RESULT = res
    out = np.concatenate(
        [r["out"].reshape(IMG_PER_CORE, O, H, W) for r in res.results], axis=0
    )
    return out.astype(np.float32)


# revision 36
# speedup vs baseline: 1.0272x; 1.0272x over previous
"""Trainium2 Bass kernel for nn_Conv2d_uint8_custom (dynamic uint8 quant + LUT conv).

Semantics (matches reference.py):
  qf = clip(round(x/scale_f) + zero_f, 0, 255)          (per-tensor dynamic quant)
  qw = clip(round(w/scale_w) + zero_w, 0, 255)
  acc[b,o,l] = sum_k lut[qf_patch, qw] = sum_k qf*qw     (lut is an exact product table)
  out = (acc - zero_f * qw_sum[o]) * scale_f * scale_w + bias[o]

Strategy (v5):
  * batch-parallel across 8 cores (2 images per core)
  * ALL quantization on host (exact fp32 replication of the reference);
    device receives pre-quantized bf16 features (ints 0..255, exact)
  * ONE feature plane per image: [flat(64ch) | up1(64ch)] in the padded
    58x58 geometry. The kh0/kh1 taps pair at K=128 for every kw via the
    column offset (0/1/2) alone; the three kh2 taps are K=64 matmuls
    that exist on BOTH halves (flat rows rt+2 / up1 rows rt+1), so two
    of them run CONCURRENTLY in disjoint PE row-groups (32x32 subarray
    tiling), and the third pairs with the adjacent tile's. Net: 4.5
    matmul slots per tile -- same PE time as the 2-plane layout with
    HALF the feature DMA (1.72MB/core) and half the weight bytes
    (one shared [128,5,128] table for both images).
  * weights ride the scalar HW DGE ring head so the first LDWEIGHTS
    gate clears ~2us earlier than the software gpsimd ring
  * PE warmup matmuls sized to bridge preamble -> first-data with no gap
  * epilogue scale+bias into bf16 (vector/scalar alternating; the last
    tile is split across both), stores batched 3+3+1 tiles per image
    with the final small store on the idle sync ring
  * host converts bf16 output back to fp32
"""

import numpy as np
import ml_dtypes
from contextlib import ExitStack

import concourse.bass as bass
import concourse.tile as tile
from concourse import bacc, mybir


def _ensure_axon_ntff_hook():
    """This image's `antenv` lacks `axon_hooks`, which bass_utils imports
    unconditionally when tracing under axon. Provide it (backed by the ctypes
    NTFF hook from trn_agent_boot when available, else None so concourse
    degrades to an untraced run)."""
    import sys, types

    if "antenv.axon_hooks" in sys.modules:
        return
    try:
        import antenv
    except ImportError:
        return
    mod = types.ModuleType("antenv.axon_hooks")
    hook = [None]
    try:
        from trn_agent_boot.trn_boot import _ntff_profile_via_ctypes

        hook[0] = _ntff_profile_via_ctypes("/opt/axon/libaxon_pjrt.so")
    except Exception:
        pass
    mod.get_axon_ntff_profile_hook = lambda: hook[0]
    mod.set_axon_ntff_profile_hook = lambda h: hook.__setitem__(0, h)
    sys.modules["antenv.axon_hooks"] = mod
    antenv.axon_hooks = mod


_ensure_axon_ntff_hook()

N_CORES = 8
B, C, H, W = 16, 64, 56, 56
O = 128
IMG_PER_CORE = B // N_CORES  # 2
L = H * W                    # 3136
HP, WP = H + 2, W + 2        # 58, 58 (zero-padded layout)
LP = HP * WP                 # 3364
TILE_ROWS = 8
NT = H // TILE_ROWS          # 7 output tiles per image
NCOL = TILE_ROWS * W         # 448 columns per tile (one PSUM bank)
N_WARM = 7                   # PE p-state warmup matmuls (big, 256 cols)
N_WARM_SMALL = 26            # trailing fine-grained warmups (64-col bridge)
WARM_COLS = 256

FP32 = mybir.dt.float32
BF16 = mybir.dt.bfloat16

# feature-plane load chunks (padded-row ranges); first small so tile 0's
# data lands as early as possible. Each DMA's completion semaphore is a
# stream of increments serialized per ring (~0.5-1us lag), so chunk
# count per ring stays low
CHUNKS = [(0, 10), (10, 26), (26, 42), (42, 58)]

_NC = None


def _build_nc():
    nc = bacc.Bacc(
        "TRN2",
        debug=False,
        enable_asserts=False,
        num_devices=N_CORES,
        enable_partition_id=False,
    )
    fq_d = nc.dram_tensor("fq", [4, 128, LP], BF16, kind="ExternalInput").ap()
    wq_d = nc.dram_tensor("wq", [128, 5, 128], BF16, kind="ExternalInput").ap()
    qp_d = nc.dram_tensor("qp", [128, 2], FP32, kind="ExternalInput").ap()
    out_d = nc.dram_tensor(
        "out", [IMG_PER_CORE, O, L], BF16, kind="ExternalOutput"
    ).ap()

    with tile.TileContext(nc) as tc:
        with ExitStack() as ctx:
            _body(ctx, tc, fq_d, wq_d, qp_d, out_d)
    nc.compile()
    return nc


def _body(ctx, tc, fq_d, wq_d, qp_d, out_d):
    nc = tc.nc
    A = mybir.AluOpType
    ID = mybir.ActivationFunctionType.Identity
    consts = ctx.enter_context(tc.tile_pool(name="consts", bufs=1))
    fpool = ctx.enter_context(tc.tile_pool(name="feat", bufs=1))
    opool = ctx.enter_context(tc.tile_pool(name="osb", bufs=6))
    ppool = ctx.enter_context(tc.tile_pool(name="acc", bufs=7, space="PSUM"))
    wpool = ctx.enter_context(tc.tile_pool(name="warm", bufs=1, space="PSUM"))

    warm = consts.tile([128, WARM_COLS], BF16)
    wq = consts.tile([128, 5, 128], BF16)
    qp = consts.tile([128, 2], FP32)
    F0 = fpool.tile([128, LP], BF16, name="F0")
    F1 = fpool.tile([128, LP], BF16, name="F1")
    M0 = fpool.tile([128, LP], BF16, name="M0")
    M1 = fpool.tile([128, LP], BF16, name="M1")

    # warm-tile memset on the vector engine (free until epilogues start)
    nc.vector.memset(warm[:], 0.0)

    def ld(eng, T, ci, plane):
        a, b = CHUNKS[ci]
        eng.dma_start(T[:, a * WP : b * WP], fq_d[plane, :, a * WP : b * WP])

    # ring layout (per-ring FIFO == stream priority): the two HW rings
    # carry PURE feature chains (M on sync, F on scalar) so each chain's
    # chunk semaphores pace with its own drain and nothing (wq/qp tiny-
    # DMA sem streams) wedges in front of the early chunks; wq/qp ride
    # the software gpsimd ring (multi-us slack before first use)
    #   sync:   M0 c0..c3, M1 c0..c3, [img0_s2, img1_s2 stores]
    #   scalar: F0 c0..c3, F1 c0..c3, [img1_s1 store]
    #   gpsimd (slow SW ring): wq, qp + bulk stores with slack
    nc.gpsimd.dma_start(wq[:], wq_d[:])
    ld(nc.sync, M0, 0, 2)
    ld(nc.scalar, F0, 0, 0)
    nc.gpsimd.dma_start(qp[:], qp_d[:])
    for ci in range(1, 4):
        ld(nc.sync, M0, ci, 2)
    for ci in range(1, 4):
        ld(nc.scalar, F0, ci, 0)
    for ci in range(4):
        ld(nc.sync, M1, ci, 3)
    for ci in range(4):
        ld(nc.scalar, F1, ci, 1)

    # PE p-state warmup: one PSUM accumulation group of dummy matmuls (no
    # WAW sems) bridges from the preamble to first-data with no idle gap.
    pw = wpool.tile([128, WARM_COLS], FP32, name="pw", tag="pw")
    n_all = N_WARM + N_WARM_SMALL
    for k in range(n_all):
        cols = WARM_COLS if k < N_WARM else 64
        nc.tensor.matmul(
            pw[:, 0:cols], warm[:, 0:128], warm[:, 0:cols],
            start=(k == 0), stop=(k == n_all - 1), skip_group_check=True,
        )

    views = {}

    def plane_view(img):
        if img not in views:
            F = F0 if img == 0 else F1
            M = M0 if img == 0 else M1
            views[img] = (
                F[:].rearrange("p (r c) -> p r c", c=WP),
                M[:].rearrange("p (r c) -> p r c", c=WP),
            )
        return views[img]

    psums = {}

    def gemm_k128(img, t, g):
        """K=128 slot g of tile t. g 0..2: (kh0 via flat + kh1 via up1)
        on the F plane at col offset g. g==3: (kh2,kw0)+(kh2,kw1) on the
        M plane ([flat | left1]) at rows rt+2. g==0 opens the tile's
        psum group. These full-array matmuls also serve as row-group
        BARRIERS around the concurrent K=64 bracket (a full-K matmul
        must wait for all in-flight row-group users, serializing
        same-bank hazards -- unbracketed chains of alternating row-group
        K=64s fault the device)."""
        fv, mv = plane_view(img)
        if g == 0:
            psums[(img, t)] = ppool.tile(
                [128, NCOL], FP32, name=f"ps{img}_{t}", tag="ps"
            )
        ps = psums[(img, t)]
        rt = TILE_ROWS * t
        if g < 3:
            src = fv[:, rt : rt + TILE_ROWS, g : g + W]
        else:
            src = mv[:, rt + 2 : rt + 10, 0:W]
        nc.tensor.matmul(
            ps[:], wq[:, g, :], src,
            start=(g == 0), stop=False, skip_group_check=True,
        )

    def tap9_lo(img, t, stop=True):
        """(kh2,kw2) via the M flat half (rows rt+2, cols+2): row-group 0."""
        _, mv = plane_view(img)
        ps = psums[(img, t)]
        rt = TILE_ROWS * t
        nc.tensor.matmul(
            ps[:], wq[0:64, 4, :], mv[0:64, rt + 2 : rt + 10, 2 : 2 + W],
            start=False, stop=stop, skip_group_check=True,
        )

    def tap9_hi(img, t, stop=True):
        """(kh2,kw2) via the M left1 half (rows rt+2, cols+1): row-group 64."""
        _, mv = plane_view(img)
        ps = psums[(img, t)]
        rt = TILE_ROWS * t
        nc.tensor.matmul(
            ps[:], wq[64:128, 4, :], mv[64:128, rt + 2 : rt + 10, 1 : 1 + W],
            start=False, stop=stop, skip_group_check=True,
        )

    # output staging + stores. The trailing batches shrink (3,3,1 tiles
    # for img0; 3,2,1,1 for img1) so the last stores are small, issue on
    # parallel rings, and their completion semaphores land asap. The two
    # final epilogues are forced onto different engines (vector then
    # scalar) so they run concurrently at PE-end.
    BATCHES = {
        0: [(0, 3, nc.gpsimd), (3, 6, nc.gpsimd), (6, 7, nc.sync)],
        1: [(0, 3, nc.gpsimd), (3, 5, nc.scalar), (5, 6, nc.sync), (6, 7, nc.sync)],
    }
    EPI_ENG = {(1, 5): "v", (1, 6): "s"}
    obuf = {}

    def epilogue(img, t, ps):
        for bi, (a, b, eng) in enumerate(BATCHES[img]):
            if a <= t < b:
                break
        nb = b - a
        if t == a:
            obuf[img] = opool.tile([128, nb * NCOL], BF16, name=f"o{img}_{bi}")
        dst = obuf[img][:, (t - a) * NCOL : (t - a + 1) * NCOL]
        kind = EPI_ENG.get((img, t), "s" if (img * NT + t) % 2 == 0 else "v")
        if kind == "s":
            nc.scalar.activation(dst, ps[:], ID, bias=qp[:, 0:1], scale=qp[:, 1:2])
        else:
            nc.vector.tensor_scalar(
                dst, ps[:], qp[:, 1:2], qp[:, 0:1], op0=A.mult, op1=A.add
            )
        if t == b - 1:
            eng.dma_start(
                out_d[img, :, a * NCOL : b * NCOL], obuf[img][:, 0 : nb * NCOL]
            )

    # Software-pipelined schedule over the 14 tiles (img-major order).
    # Per tile: 4 K=128 slots + half of one K=64 bracket. The two
    # (kh2,kw2) K=64 matmuls of a tile pair run as ONE concurrent
    # cross-bank bracket [rg0 on psX || rg64 on psY], bracketed by
    # full-K=128 matmuls of a LATER tile as row-group barriers.
    TILES = [(0, t) for t in range(NT)] + [(1, t) for t in range(NT)]

    def G(i, g):
        if i < len(TILES):
            gemm_k128(TILES[i][0], TILES[i][1], g)

    G(0, 0); G(0, 1); G(0, 2); G(0, 3)
    G(1, 0)
    for k in range(7):
        X, Y = TILES[2 * k], TILES[2 * k + 1]
        G(2 * k + 1, 1); G(2 * k + 1, 2); G(2 * k + 1, 3)
        tap9_lo(*X)                              # bracket: rg0 on psX
        tap9_hi(*Y)                              #          rg64 on psY
        G(2 * k + 2, 0)                          # barrier + open next tile
        epilogue(*X, psums[X])
        epilogue(*Y, psums[Y])
        G(2 * k + 2, 1); G(2 * k + 2, 2); G(2 * k + 2, 3)
        G(2 * k + 3, 0)


def _prep_host(x, weight, bias):
    """Exact fp32 replication of the reference's quantization arithmetic
    (numpy and jax-on-cpu are both IEEE fp32, round-half-even), then pack
    the padded bf16 feature planes [flat|up1], the shared bf16 weight
    table, and the folded epilogue scale/bias."""
    f = np.float32
    mx, mn = f(x.max()), f(x.min())
    scale_f = f((mx - mn) / f(255.0))
    zero_f = f(-np.round(mn / scale_f))
    qf = np.clip(
        np.round(x.astype(np.float32) / scale_f) + zero_f, 0.0, 255.0
    ).astype(ml_dtypes.bfloat16)  # exact small ints

    mw, nw = f(weight.max()), f(weight.min())
    scale_w = f((mw - nw) / f(255.0))
    zero_w = f(-np.round(nw / scale_w))
    qw = np.clip(
        np.round(weight.astype(np.float32) / scale_w) + zero_w, 0.0, 255.0
    ).astype(np.float32)  # exact small ints

    s_tot = f(scale_f * scale_w)
    qw_sum = qw.reshape(O, -1).sum(axis=1, dtype=np.float64)
    bias_eff = (
        bias.astype(np.float64) - np.float64(zero_f) * qw_sum * np.float64(s_tot)
    ).astype(np.float32)
    qp = np.zeros((128, 2), np.float32)
    qp[:, 0] = bias_eff
    qp[:, 1] = s_tot

    # padded features [B, C, 58*58] + the up1 (flat +58) shifted variant
    pad = np.zeros((B, C, HP, WP), ml_dtypes.bfloat16)
    pad[:, :, 1 : 1 + H, 1 : 1 + W] = qf
    flat = pad.reshape(B, C, LP)
    shU = np.zeros_like(flat)
    shU[:, :, : LP - WP] = flat[:, :, WP:]

    shL = np.zeros_like(flat)
    shL[:, :, : LP - 1] = flat[:, :, 1:]

    # per-core planes [4, 128, LP]: F = [flat|up1] (kh0/kh1 pairs at col
    # offsets 0/1/2), M = [flat|left1] (kh2 pairs at rows rt+2)
    fq_cores = []
    for c in range(N_CORES):
        i0, i1 = 2 * c, 2 * c + 1
        p_f0 = np.concatenate([flat[i0], shU[i0]], axis=0)
        p_f1 = np.concatenate([flat[i1], shU[i1]], axis=0)
        p_m0 = np.concatenate([flat[i0], shL[i0]], axis=0)
        p_m1 = np.concatenate([flat[i1], shL[i1]], axis=0)
        fq_cores.append(
            np.ascontiguousarray(np.stack([p_f0, p_f1, p_m0, p_m1]))
        )

    # shared weight table [128 (K), 5 (slot), 128 (O)]:
    #   s0: (kh0,kw0) lo | (kh1,kw0) hi     s1: (kh0,kw1) | (kh1,kw1)
    #   s2: (kh0,kw2) | (kh1,kw2)           s3: (kh2,kw0) lo | (kh2,kw1) hi
    #   s4: (kh2,kw2) on both halves (tiles use one half each, paired)
    qwT = qw.transpose(2, 3, 1, 0)  # [kh, kw, C, O]
    wqa = np.zeros((128, 5, 128), np.float32)
    for kw in range(3):
        wqa[0:64, kw] = qwT[0, kw]
        wqa[64:128, kw] = qwT[1, kw]
    wqa[0:64, 3] = qwT[2, 0]
    wqa[64:128, 3] = qwT[2, 1]
    wqa[0:64, 4] = qwT[2, 2]
    wqa[64:128, 4] = qwT[2, 2]
    return fq_cores, wqa.astype(ml_dtypes.bfloat16), qp


def build():
    global _NC
    if _NC is None:
        _NC = _build_nc()
    return _NC


LAST_RESULT = None


def kernel(x, weight, bias, lut):
    global LAST_RESULT
    from concourse.bass_utils import run_bass_kernel_spmd

    x = np.asarray(x, dtype=np.float32)
    weight = np.asarray(weight, dtype=np.float32)
    bias = np.asarray(bias, dtype=np.float32)

    fq_cores, wq, qp = _prep_host(x, weight, bias)
    nc = build()
    in_maps = [
        {"fq": fq_cores[c], "wq": wq, "qp": qp} for c in range(N_CORES)
    ]

    res = run_bass_kernel_spmd(nc, in_maps, core_ids=list(range(N_CORES)))
    LAST_RESULT = res
    out = np.concatenate(
        [r["out"].reshape(IMG_PER_CORE, O, H, W) for r in res.results], axis=0
    )
    return out.astype(np.float32)
